# revision 2
# baseline (speedup 1.0000x reference)
"""Distributed Evoformer block on 8 Trainium2 NeuronCores.

Strategy (sequence/row parallel, per the sharding hint):
  - pair tensor sharded along the first residue axis: 32 rows/core.
  - msa replicated (8.4 MB); msa-track outputs sharded along the residue
    axis so each core does 1/8 of the work.
  - Cross-row dependencies handled with exactly three sizeable
    collectives:
      * tmo (outgoing triangle mult):  all-gather of the gated `b`
        projection (each core contracts its own `a` rows against the
        full `b`).
      * tmi (incoming triangle mult):  contraction runs over the row
        axis, so each core computes a full-size partial sum from its
        own rows; one psum-scatter returns the row shard.
      * pa2 (transposed grid attention): all-gather of the normalized
        pair so each core builds full k/v in transposed space; queries,
        gating and outputs stay row-local, so the output needs no
        communication.
    Plus one tiny all-gather of the pa1 attention bias ([4,256,256]).
  - Parameters replicated.
"""

import numpy as np
import jax
import jax.numpy as jnp
from jax.sharding import Mesh, PartitionSpec as P

try:
    from jax.experimental.shard_map import shard_map
except Exception:  # newer jax
    from jax import shard_map

C_MSA, C_PAIR = 64, 128
H_PAIR, H_MSA = 4, 8
N_SEQ, N_RES = 128, 256
NCORES = 8
ROWS = N_RES // NCORES  # 32 pair rows per core


def _ln(x, g, b):
    m = jnp.mean(x, axis=-1, keepdims=True)
    v = jnp.mean(jnp.square(x - m), axis=-1, keepdims=True)
    return (x - m) * jax.lax.rsqrt(v + 1e-5) * g + b


def _transition(x, g, b, w1, w2):
    y = _ln(x, g, b)
    a, bb = jnp.split(y @ w1, 2, axis=-1)
    return (jax.nn.silu(a) * bb) @ w2


def _shard_fn(msa, pair_sh, msa_mask, pair_mask, p):
    """Runs on one core. pair_sh: [32, 256, 128] rows of the pair tensor."""
    idx = jax.lax.axis_index("x")
    r0 = idx * ROWS

    def rows(full, axis=0):
        return jax.lax.dynamic_slice_in_dim(full, r0, ROWS, axis=axis)

    # ---- OuterProductMean (pair rows i in my chunk; msa replicated) ----
    x = _ln(msa, p["opm_ln_g"], p["opm_ln_b"])
    m = msa_mask[..., None]
    a_full = (x @ p["opm_left_w"]) * m   # [s, i, 32]
    b_full = (x @ p["opm_right_w"]) * m  # [s, j, 32]
    a_sh = rows(a_full, axis=1)          # [s, 32rows, 32]
    outer = jnp.einsum("sic,sje->ijce", a_sh, b_full)
    opm = jnp.einsum("ijce,cef->ijf", outer, p["opm_out_w"]) + p["opm_out_b"]
    norm = jnp.einsum("si,sj->ij", rows(msa_mask, axis=1), msa_mask)[..., None]
    pair_sh = pair_sh + opm / (1e-3 + norm)

    # ---- MSA row attention (msa columns i in my chunk) ----
    act = _ln(msa, p["ma_act_ln_g"], p["ma_act_ln_b"])
    pz = _ln(pair_sh, p["ma_pair_ln_g"], p["ma_pair_ln_b"])
    logits = jnp.einsum("ijc,ch->hij", pz, p["ma_pair_logits_w"])  # [h, 32, 256]
    logits = logits + 1e9 * (jnp.max(msa_mask, axis=0) - 1.0)[None, None, :]
    wts = jax.nn.softmax(logits, axis=-1)
    d = C_MSA // H_MSA
    v = (act @ p["ma_v_w"]).reshape(N_SEQ, N_RES, H_MSA, d)
    o = jnp.einsum("hij,sjhd->sihd", wts, v).reshape(N_SEQ, ROWS, C_MSA)
    act_sh = rows(act, axis=1)
    o = o * jax.nn.sigmoid(act_sh @ p["ma_gating_w"])
    msa_sh = rows(msa, axis=1) + o @ p["ma_out_w"]

    # ---- MSA transition ----
    msa_sh = msa_sh + _transition(
        msa_sh, p["mt_ln_g"], p["mt_ln_b"], p["mt_w1"], p["mt_w2"]
    )

    mask_rows = rows(pair_mask, axis=0)[..., None]  # [32, 256, 1]

    # ---- Triangle multiplication, outgoing ----
    x = _ln(pair_sh, p["tmo_ln_g"], p["tmo_ln_b"])
    proj = (x @ p["tmo_proj_w"]) * mask_rows
    proj = proj * jax.nn.sigmoid(x @ p["tmo_gate_w"])
    a_sh, b_sh = jnp.split(proj, 2, axis=-1)  # [32, 256, 128] each
    b_all = jax.lax.all_gather(
        b_sh.astype(jnp.bfloat16), "x", axis=0, tiled=True
    ).astype(jnp.float32)  # [256, 256, 128]
    z = jnp.einsum("ikc,jkc->ijc", a_sh, b_all)
    z = _ln(z, p["tmo_center_ln_g"], p["tmo_center_ln_b"]) @ p["tmo_out_w"]
    pair_sh = pair_sh + z * jax.nn.sigmoid(x @ p["tmo_gating_w"])

    # ---- Triangle multiplication, incoming ----
    x = _ln(pair_sh, p["tmi_ln_g"], p["tmi_ln_b"])
    proj = (x @ p["tmi_proj_w"]) * mask_rows
    proj = proj * jax.nn.sigmoid(x @ p["tmi_gate_w"])
    a_sh, b_sh = jnp.split(proj, 2, axis=-1)  # rows are the contracted k axis
    z_part = jnp.einsum("kjc,kic->ijc", a_sh, b_sh)  # [256, 256, 128] partial
    z = jax.lax.psum_scatter(z_part, "x", scatter_dimension=0, tiled=True)
    z = _ln(z, p["tmi_center_ln_g"], p["tmi_center_ln_b"]) @ p["tmi_out_w"]
    pair_sh = pair_sh + z * jax.nn.sigmoid(x @ p["tmi_gating_w"])

    dp = C_PAIR // H_PAIR
    scale = dp ** -0.5

    # ---- Grid attention 1 (row-wise, batch = my rows) ----
    x = _ln(pair_sh, p["pa1_ln_g"], p["pa1_ln_b"])
    bias_sh = jnp.einsum("qkc,ch->hqk", x, p["pa1_bias_w"])  # [4, 32, 256]
    bias = jax.lax.all_gather(bias_sh, "x", axis=1, tiled=True)  # [4, 256, 256]
    shp = (ROWS, N_RES, H_PAIR, dp)
    q = (x @ p["pa1_q_w"]).reshape(shp) * scale
    k = (x @ p["pa1_k_w"]).reshape(shp)
    v = (x @ p["pa1_v_w"]).reshape(shp)
    logits = jnp.einsum("bqhd,bkhd->bhqk", q, k) + bias[None]
    logits = logits + (1e9 * (mask_rows[:, :, 0] - 1.0))[:, None, None, :]
    wts = jax.nn.softmax(logits, axis=-1)
    o = jnp.einsum("bhqk,bkhd->bqhd", wts, v).reshape(ROWS, N_RES, C_PAIR)
    o = o * jax.nn.sigmoid(x @ p["pa1_gating_w"])
    pair_sh = pair_sh + o @ p["pa1_out_w"]

    # ---- Grid attention 2 (column-wise; shard over the query axis q,
    #      which is the original row axis, so outputs stay row-local) ----
    x = _ln(pair_sh, p["pa2_ln_g"], p["pa2_ln_b"])  # [32(q), 256, 128]
    bias_sh = jnp.einsum("qkc,ch->hqk", x, p["pa2_bias_w"])  # [4, 32(q), 256]
    x_all = jax.lax.all_gather(
        x.astype(jnp.bfloat16), "x", axis=0, tiled=True
    ).astype(jnp.float32)  # [256, 256, 128] full normalized pair
    xt = jnp.swapaxes(x_all, 0, 1)       # [256(b), 256(rows), 128]
    xq = jnp.swapaxes(x, 0, 1)           # [256(b), 32(q rows), 128]
    q = (xq @ p["pa2_q_w"]).reshape(N_RES, ROWS, H_PAIR, dp) * scale
    k = (xt @ p["pa2_k_w"]).reshape(N_RES, N_RES, H_PAIR, dp)
    v = (xt @ p["pa2_v_w"]).reshape(N_RES, N_RES, H_PAIR, dp)
    # bias term: logits[b,h,q,k] += bias_sh[h,q,k] (same for every batch b)
    logits = jnp.einsum("bqhd,bkhd->bhqk", q, k) + bias_sh[None]  # [256,4,32,256]
    logits = logits + (1e9 * (pair_mask - 1.0))[:, None, None, :]
    wts = jax.nn.softmax(logits, axis=-1)
    o = jnp.einsum("bhqk,bkhd->bqhd", wts, v).reshape(N_RES, ROWS, C_PAIR)
    o = o * jax.nn.sigmoid(xq @ p["pa2_gating_w"])
    o = o @ p["pa2_out_w"]
    pair_sh = pair_sh + jnp.swapaxes(o, 0, 1)  # back to [32(rows), 256, 128]

    # ---- Pair transition ----
    pair_sh = pair_sh + _transition(
        pair_sh, p["pt_ln_g"], p["pt_ln_b"], p["pt_w1"], p["pt_w2"]
    )

    return msa_sh, pair_sh


_COMPILED = {}


def _get_fn():
    if "fn" not in _COMPILED:
        devs = jax.devices()[:NCORES]
        mesh = Mesh(np.array(devs), ("x",))
        fn = jax.jit(
            shard_map(
                _shard_fn,
                mesh=mesh,
                in_specs=(P(), P("x"), P(), P(), P()),
                out_specs=(P(None, "x"), P("x")),
            )
        )
        _COMPILED["fn"] = fn
    return _COMPILED["fn"]


def kernel(msa, pair, msa_mask, pair_mask, params):
    fn = _get_fn()
    p = {k: jnp.asarray(np.asarray(v), jnp.float32) for k, v in params.items()}
    out_msa, out_pair = fn(
        jnp.asarray(np.asarray(msa), jnp.float32),
        jnp.asarray(np.asarray(pair), jnp.float32),
        jnp.asarray(np.asarray(msa_mask), jnp.float32),
        jnp.asarray(np.asarray(pair_mask), jnp.float32),
        p,
    )
    return np.asarray(out_msa), np.asarray(out_pair)
